# revision 27
# baseline (speedup 1.0000x reference)
"""Distributed inverse real vector SHT on 8 Trainium2 NeuronCores.

Decomposition (2D polar x azimuth, per the original model's parallelism):
  Stage 1 (sharded over m): for each m, the four Legendre contractions are
  two accumulating matmuls  Z[m] = X1[m]^T @ dT0[m] + X2[m]^T @ dT1[m]
  where the 128 columns of X1/X2 pack the four (re/im x s/t) input blocks
  with signs arranged so the PSUM accumulation directly produces
  rows [srl, sim, tim, trl] (no combine step).
  All-to-all (split 3 ways in m so it overlaps stage 1's tail): re-shard
  from m-split to nlat(k)-split.
  Stage 2 (sharded over k): transpose Z on the PE (c2 partition -> m
  partition; regular matmuls against the identity, evicted 4-per-psum-bank
  with one wide DVE copy), then the irfft is a real matmul against
  precomputed cos/sin tables contracting over (re/im, m).
"""
import sys
import os
sys.path.insert(0, '/opt/trn_rl_repo')
import numpy as np
import ml_dtypes

from concourse import bacc, tile, mybir, masks
from concourse.bass_utils import run_bass_kernel_spmd

B, C, L, M, K, N = 1, 32, 361, 361, 361, 720
NC = 8
MP = 368                    # m padded to 8*46
MC = MP // NC               # 46 m's per core
MSEG = [(0, 16), (16, 16), (32, 14)]   # per-core m split (3 collectives)
KC = 46                     # k's per core
KPP = 368
LP = 384                    # l padded to 3*128
LCH = 3
NG = (KC + 3) // 4          # 12 kj-groups (last has 2)
BF16 = ml_dtypes.bfloat16

_CACHE = {}


def _build():
    nc = bacc.Bacc("TRN2", target_bir_lowering=False, debug=False,
                   num_devices=NC)
    xsh = nc.dram_tensor("xsh", [128, MC, 2, LCH, 128], mybir.dt.bfloat16,
                         kind="ExternalInput")
    dsh = nc.dram_tensor("dsh", [128, MC, 2, LCH, KPP], mybir.dt.bfloat16,
                         kind="ExternalInput")
    ctab = nc.dram_tensor("ctab", [128, 2, LCH, N], mybir.dt.bfloat16,
                          kind="ExternalInput")
    outsh = nc.dram_tensor("outsh", [2, C, KC, N], mybir.dt.bfloat16,
                           kind="ExternalOutput")

    m_blocks = []
    m0 = 0
    for cnt in (4, 4, 8, 8, 8, 8, 6):
        m_blocks.append((m0, cnt))
        m0 += cnt
    assert m0 == MC

    with tile.TileContext(nc) as tc:
        with tc.tile_pool(name="dram", bufs=1, space="DRAM") as dram, \
             tc.tile_pool(name="const", bufs=1) as constp:
            a2a_in = [dram.tile([NC, 128, mw, KC], mybir.dt.bfloat16,
                                name=f"a2ain{s}")
                      for s, (ms, mw) in enumerate(MSEG)]
            a2a_out = [dram.tile([NC, 128, mw, KC], mybir.dt.bfloat16,
                                 name=f"a2aout{s}")
                       for s, (ms, mw) in enumerate(MSEG)]
            ident = constp.tile([128, 128], mybir.dt.bfloat16)
            masks.make_identity(nc, ident[:])

            # ---------------- stage 1: Legendre contractions (m-sharded)
            with tc.tile_pool(name="s1", bufs=2) as s1, \
                 tc.tile_pool(name="zs", bufs=1) as zs, \
                 tc.tile_pool(name="ps1", bufs=4, space="PSUM") as ps1:
                # kg-major staging: a2a writes contiguous on both sides
                zst = [zs.tile([128, NC, mw, KC], mybir.dt.bfloat16,
                               name=f"zst{s}")
                       for s, (ms, mw) in enumerate(MSEG)]
                for (m0, cnt) in m_blocks:
                    dt = s1.tile([128, 8, 2, LCH, KPP], mybir.dt.bfloat16,
                                 tag="dt")
                    xt = s1.tile([128, 8, 2, LCH, 128], mybir.dt.bfloat16,
                                 tag="xt")
                    nc.sync.dma_start(out=dt[:, :cnt], in_=dsh[:, m0:m0+cnt])
                    nc.sync.dma_start(out=xt[:, :cnt], in_=xsh[:, m0:m0+cnt])
                    for ml in range(cnt):
                        zt = ps1.tile([128, KPP], mybir.dt.float32, tag="zt")
                        for lc in range(LCH):
                            for w in range(2):
                                nc.tensor.matmul(
                                    out=zt[:],
                                    lhsT=xt[:, ml, w, lc, :],
                                    rhs=dt[:, ml, w, lc, :],
                                    start=(lc == 0 and w == 0),
                                    stop=(lc == LCH - 1 and w == 1),
                                )
                        mg = m0 + ml
                        seg = 0 if mg < 16 else (1 if mg < 32 else 2)
                        nc.vector.tensor_copy(
                            out=zst[seg][:, :, mg - MSEG[seg][0], :],
                            in_=zt[:].rearrange("p (g k) -> p g k", k=KC))
                    # fire each segment's a2a writes as soon as complete
                    for s, (ms, mw) in enumerate(MSEG):
                        if m0 + cnt == ms + mw:
                            for kg in range(NC):
                                nc.scalar.dma_start(
                                    out=a2a_in[s][kg], in_=zst[s][:, kg])

            for s in range(len(MSEG)):
                nc.gpsimd.collective_compute(
                    "AllToAll", mybir.AluOpType.bypass,
                    replica_groups=[list(range(NC))],
                    ins=[a2a_in[s].opt()], outs=[a2a_out[s].opt()],
                )

            # ---------------- stage 2: irfft as matmul (k-sharded)
            with tc.tile_pool(name="s2", bufs=1) as s2, \
                 tc.tile_pool(name="acc", bufs=1) as accp, \
                 tc.tile_pool(name="ob", bufs=4) as ob:
                ct = s2.tile([128, 2, LCH, N], mybir.dt.bfloat16, tag="ct")
                nc.sync.dma_start(out=ct[:], in_=ctab[:])
                ztmp = []
                for s, (ms, mw) in enumerate(MSEG):
                    t = s2.tile([128, NC * mw, KC], mybir.dt.bfloat16,
                                name=f"ztmp{s}")
                    nc.scalar.dma_start(
                        out=t.rearrange("c (b m) k -> c b m k", m=mw),
                        in_=a2a_out[s].rearrange("b c m k -> c b m k"))
                    ztmp.append((t, NC * mw))

                ztr = []
                for mc in range(3):
                    t = s2.tile([128, 4, NG, 4, 32], mybir.dt.bfloat16,
                                tag=f"ztr{mc}", name=f"ztr{mc}")
                    nc.vector.memset(t[:, :, NG-1, 2:4, :], 0.0)
                    ztr.append(t)

                # transpose c2->m via regular matmuls against the identity;
                # 4 transposes per psum bank, one wide eviction copy
                def transposes(ps2tp, mc):
                    src, mcnt = ztmp[mc]
                    t = ztr[mc]
                    for g in range(NG):
                        kw = min(4, KC - g * 4)
                        ptb = ps2tp.tile([128, 512], mybir.dt.float32,
                                         tag="ptb")
                        for kk in range(kw):
                            nc.tensor.matmul(
                                out=ptb[:mcnt, kk*128:(kk+1)*128],
                                lhsT=src[:, :mcnt, g*4 + kk],
                                rhs=ident[:], start=True, stop=True)
                        nc.vector.tensor_copy(
                            out=t[:mcnt, :, g, 0:kw, :],
                            in_=ptb[:mcnt, 0:kw*128].rearrange(
                                "m (k b c) -> m b k c", b=4, c=32))

                # comp 0 (s): srl rows (b=0) w/ Cre, sim rows (b=1) w/ Cim
                # comp 1 (t): trl rows (b=3) w/ Cre, tim rows (b=2) w/ Cim
                comp_seq = [((0, 0), (1, 1)), ((3, 0), (2, 1))]
                acc = [accp.tile([128, N], mybir.dt.bfloat16,
                                 name=f"acc{i}") for i in range(2 * NG)]
                with tc.tile_pool(name="ps2tp", bufs=4,
                                  space="PSUM") as ps2tp, \
                     tc.tile_pool(name="ps2po", bufs=2,
                                  space="PSUM") as ps2po:
                    transposes(ps2tp, 0)
                    transposes(ps2tp, 1)
                    # phase 1: partial DFT over m-chunks 0,1 (overlaps the
                    # in-flight last collective)
                    for comp in range(2):
                        for g in range(NG):
                            po0 = ps2po.tile([128, 360], mybir.dt.float32,
                                             tag="po0")
                            po1 = ps2po.tile([128, 360], mybir.dt.float32,
                                             tag="po1")
                            i = 0
                            for (b, reim) in comp_seq[comp]:
                                for mc in (0, 1):
                                    mcnt = ztmp[mc][1]
                                    lhsT = ztr[mc][:mcnt, b, g]
                                    nc.tensor.matmul(
                                        out=po0[:], lhsT=lhsT,
                                        rhs=ct[:mcnt, reim, mc, 0:360],
                                        start=(i == 0), stop=(i == 3))
                                    nc.tensor.matmul(
                                        out=po1[:], lhsT=lhsT,
                                        rhs=ct[:mcnt, reim, mc, 360:720],
                                        start=(i == 0), stop=(i == 3))
                                    i += 1
                            a = acc[comp * NG + g]
                            nc.vector.tensor_copy(out=a[:, 0:360],
                                                  in_=po0[:])
                            nc.vector.tensor_copy(out=a[:, 360:720],
                                                  in_=po1[:])
                    transposes(ps2tp, 2)
                    # phase 2: finish with m-chunk 2; phase-1 partials
                    # re-enter PSUM via a matmul against the identity
                    mcnt2 = ztmp[2][1]
                    for comp in range(2):
                        for g in range(NG):
                            po0 = ps2po.tile([128, 360], mybir.dt.float32,
                                             tag="po0")
                            po1 = ps2po.tile([128, 360], mybir.dt.float32,
                                             tag="po1")
                            a = acc[comp * NG + g]
                            nc.tensor.matmul(
                                out=po0[:], lhsT=ident[:],
                                rhs=a[:, 0:360], start=True, stop=False)
                            nc.tensor.matmul(
                                out=po1[:], lhsT=ident[:],
                                rhs=a[:, 360:720], start=True, stop=False)
                            i = 0
                            for (b, reim) in comp_seq[comp]:
                                lhsT = ztr[2][:mcnt2, b, g]
                                nc.tensor.matmul(
                                    out=po0[:], lhsT=lhsT,
                                    rhs=ct[:mcnt2, reim, 2, 0:360],
                                    start=False, stop=(i == 1))
                                nc.tensor.matmul(
                                    out=po1[:], lhsT=lhsT,
                                    rhs=ct[:mcnt2, reim, 2, 360:720],
                                    start=False, stop=(i == 1))
                                i += 1
                            osb = ob.tile([128, N], mybir.dt.bfloat16,
                                          tag="osb")
                            nc.vector.tensor_copy(out=osb[:, 0:360],
                                                  in_=po0[:])
                            nc.vector.tensor_copy(out=osb[:, 360:720],
                                                  in_=po1[:])
                            kw = min(4, KC - g * 4)
                            # psum rows are (kj, c); alternate HWDGE rings
                            eng = nc.sync if (g % 2 == 0) else nc.scalar
                            eng.dma_start(
                                out=outsh[comp].rearrange(
                                    "c k n -> k c n")[g*4:g*4+kw],
                                in_=osb[:kw*C],
                            )
    nc.compile()
    return nc


def _m_perm():
    """Row order of the m axis as seen by stage 2 (segment-major)."""
    perm = []
    for (ms, mw) in MSEG:
        perm += [mb * MC + ms + ml for mb in range(NC) for ml in range(mw)]
    return np.array(perm)


def _host_prep(x_re, x_im, d0, d1):
    xr0, xr1 = x_re[0, :, 0], x_re[0, :, 1]   # (32, L, M)
    xi0, xi1 = x_im[0, :, 0], x_im[0, :, 1]

    def mkx(blocks):
        x = np.concatenate(blocks, axis=0)            # (128, L, M)
        x = np.transpose(x, (2, 1, 0))                # (M, L, 128)
        xp = np.zeros((MP, LP, 128), BF16)
        xp[:M, :L] = x
        return xp
    X1 = mkx([xr0, xi0, -xi1, -xr1])
    X2 = mkx([-xi1, xr1, xr0, -xi0])
    # xsh[core][p, ml, which, lc, c'] = X{which}[core*MC+ml, lc*128+p, c']
    xv = np.stack([X1, X2], axis=1)                   # (MP, 2, LP, 128)
    xv = xv.reshape(NC, MC, 2, LCH, 128, 128)         # (i, ml, w, lc, p, c)
    xv = np.ascontiguousarray(xv.transpose(0, 4, 1, 2, 3, 5))

    def mkd(d):
        dp = np.zeros((MP, LP, KPP), BF16)
        dp[:M, :L, :K] = np.transpose(d, (0, 2, 1))
        return dp
    D0, D1 = mkd(d0), mkd(d1)
    dv = np.stack([D0, D1], axis=1)                   # (MP, 2, LP, KPP)
    dv = dv.reshape(NC, MC, 2, LCH, 128, KPP)
    dv = np.ascontiguousarray(dv.transpose(0, 4, 1, 2, 3, 5))

    m = np.arange(MP, dtype=np.float64)[:, None]
    n = np.arange(N, dtype=np.float64)[None, :]
    th = 2.0 * np.pi * (m * n) / N
    w = np.full((MP, 1), 2.0); w[0] = 1.0; w[360] = 1.0; w[361:] = 0.0
    Cre = (w * np.cos(th)).astype(np.float32)
    Cim = (-w * np.sin(th)).astype(np.float32)
    Cim[0] = 0.0; Cim[360] = 0.0; Cim[361:] = 0.0
    cv = np.stack([Cre, Cim], axis=1)                 # (MP, 2, N)
    cv = cv[_m_perm()]                                # stage-2 m order
    cv = np.concatenate(
        [cv, np.zeros((LCH * 128 - MP, 2, N), np.float32)], axis=0)
    cv = cv.reshape(LCH, 128, 2, N)
    cv = np.ascontiguousarray(cv.transpose(1, 2, 0, 3)).astype(BF16)
    return xv, dv, cv


def kernel(x_re, x_im, d0, d1):
    if "nc" not in _CACHE:
        _CACHE["nc"] = _build()
    nc = _CACHE["nc"]

    xv, dv, cv = _host_prep(np.asarray(x_re), np.asarray(x_im),
                            np.asarray(d0), np.asarray(d1))
    in_maps = [{"xsh": xv[i], "dsh": dv[i], "ctab": cv} for i in range(NC)]
    res = run_bass_kernel_spmd(nc, in_maps, list(range(NC)))

    out = np.empty((B, C, 2, K, N), np.float32)
    for i in range(NC):
        k0 = i * KC
        k1 = min(K, k0 + KC)
        o = res.results[i]["outsh"].astype(np.float32)  # [2, C, KC, N]
        out[0, :, 0, k0:k1] = o[0, :, :k1-k0]
        out[0, :, 1, k0:k1] = o[1, :, :k1-k0]
    return out


# revision 28
# speedup vs baseline: 1.0123x; 1.0123x over previous
"""Distributed inverse real vector SHT on 8 Trainium2 NeuronCores.

Decomposition (2D polar x azimuth, per the original model's parallelism):
  Stage 1 (sharded over m): for each m, the four Legendre contractions are
  two accumulating matmuls  Z[m] = X1[m]^T @ dT0[m] + X2[m]^T @ dT1[m]
  where the 128 columns of X1/X2 pack the four (re/im x s/t) input blocks
  with signs arranged so the PSUM accumulation directly produces
  rows [srl, sim, tim, trl] (no combine step).
  All-to-all (split 3 ways in m so it overlaps stage 1's tail): re-shard
  from m-split to nlat(k)-split.
  Stage 2 (sharded over k): transpose Z on the PE (c2 partition -> m
  partition; regular matmuls against the identity, evicted 4-per-psum-bank
  with one wide DVE copy), then the irfft is a real matmul against
  precomputed cos/sin tables contracting over (re/im, m).
"""
import sys
import os
sys.path.insert(0, '/opt/trn_rl_repo')
import numpy as np
import ml_dtypes

from concourse import bacc, tile, mybir, masks
from concourse.bass_utils import run_bass_kernel_spmd

B, C, L, M, K, N = 1, 32, 361, 361, 361, 720
NC = 8
MP = 368                    # m padded to 8*46
MC = MP // NC               # 46 m's per core
MSEG = [(0, 16), (16, 16), (32, 14)]   # per-core m split (3 collectives)
KC = 46                     # k's per core
KPP = 368
LP = 384                    # l padded to 3*128
LCH = 3
NG = (KC + 3) // 4          # 12 kj-groups (last has 2)
BF16 = ml_dtypes.bfloat16

_CACHE = {}


def _build():
    nc = bacc.Bacc("TRN2", target_bir_lowering=False, debug=False,
                   num_devices=NC)
    xsh = nc.dram_tensor("xsh", [128, MC, 2, LCH, 128], mybir.dt.bfloat16,
                         kind="ExternalInput")
    dsh = nc.dram_tensor("dsh", [128, MC, 2, LCH, KPP], mybir.dt.bfloat16,
                         kind="ExternalInput")
    ctab = nc.dram_tensor("ctab", [128, 2, LCH, N], mybir.dt.bfloat16,
                          kind="ExternalInput")
    outsh = nc.dram_tensor("outsh", [2, C, KC, N], mybir.dt.bfloat16,
                           kind="ExternalOutput")

    m_blocks = []
    m0 = 0
    for cnt in (4, 4, 8, 8, 8, 8, 6):
        m_blocks.append((m0, cnt))
        m0 += cnt
    assert m0 == MC

    with tile.TileContext(nc) as tc:
        with tc.tile_pool(name="dram", bufs=1, space="DRAM") as dram, \
             tc.tile_pool(name="const", bufs=1) as constp:
            a2a_in = [dram.tile([NC, 128, mw, KC], mybir.dt.bfloat16,
                                name=f"a2ain{s}")
                      for s, (ms, mw) in enumerate(MSEG)]
            a2a_out = [dram.tile([NC, 128, mw, KC], mybir.dt.bfloat16,
                                 name=f"a2aout{s}")
                       for s, (ms, mw) in enumerate(MSEG)]
            ident = constp.tile([128, 128], mybir.dt.bfloat16)
            masks.make_identity(nc, ident[:])

            # ---------------- stage 1: Legendre contractions (m-sharded)
            with tc.tile_pool(name="s1", bufs=2) as s1, \
                 tc.tile_pool(name="zs", bufs=1) as zs, \
                 tc.tile_pool(name="ps1", bufs=4, space="PSUM") as ps1:
                # kg-major staging: a2a writes contiguous on both sides
                zst = [zs.tile([128, NC, mw, KC], mybir.dt.bfloat16,
                               name=f"zst{s}")
                       for s, (ms, mw) in enumerate(MSEG)]
                for (m0, cnt) in m_blocks:
                    dt = s1.tile([128, 8, 2, LCH, KPP], mybir.dt.bfloat16,
                                 tag="dt")
                    xt = s1.tile([128, 8, 2, LCH, 128], mybir.dt.bfloat16,
                                 tag="xt")
                    nc.sync.dma_start(out=dt[:, :cnt], in_=dsh[:, m0:m0+cnt])
                    nc.sync.dma_start(out=xt[:, :cnt], in_=xsh[:, m0:m0+cnt])
                    for ml in range(cnt):
                        zt = ps1.tile([128, KPP], mybir.dt.float32, tag="zt")
                        for lc in range(LCH):
                            for w in range(2):
                                nc.tensor.matmul(
                                    out=zt[:],
                                    lhsT=xt[:, ml, w, lc, :],
                                    rhs=dt[:, ml, w, lc, :],
                                    start=(lc == 0 and w == 0),
                                    stop=(lc == LCH - 1 and w == 1),
                                )
                        mg = m0 + ml
                        seg = 0 if mg < 16 else (1 if mg < 32 else 2)
                        nc.vector.tensor_copy(
                            out=zst[seg][:, :, mg - MSEG[seg][0], :],
                            in_=zt[:].rearrange("p (g k) -> p g k", k=KC))
                    # fire each segment's a2a writes as soon as complete
                    for s, (ms, mw) in enumerate(MSEG):
                        if m0 + cnt == ms + mw:
                            for kg in range(NC):
                                nc.scalar.dma_start(
                                    out=a2a_in[s][kg], in_=zst[s][:, kg])

            for s in range(len(MSEG)):
                nc.gpsimd.collective_compute(
                    "AllToAll", mybir.AluOpType.bypass,
                    replica_groups=[list(range(NC))],
                    ins=[a2a_in[s].opt()], outs=[a2a_out[s].opt()],
                )

            # ---------------- stage 2: irfft as matmul (k-sharded)
            with tc.tile_pool(name="s2", bufs=1) as s2, \
                 tc.tile_pool(name="ob", bufs=4) as ob:
                ct = s2.tile([128, 2, LCH, N], mybir.dt.bfloat16, tag="ct")
                nc.sync.dma_start(out=ct[:], in_=ctab[:])
                ztmp = []
                for s, (ms, mw) in enumerate(MSEG):
                    t = s2.tile([128, NC * mw, KC], mybir.dt.bfloat16,
                                name=f"ztmp{s}")
                    nc.scalar.dma_start(
                        out=t.rearrange("c (b m) k -> c b m k", m=mw),
                        in_=a2a_out[s].rearrange("b c m k -> c b m k"))
                    ztmp.append((t, NC * mw))

                # transpose c2->m via regular matmuls against the identity;
                # 4 transposes per psum bank, one wide eviction copy
                ztr = []
                with tc.tile_pool(name="ps2tp", bufs=6,
                                  space="PSUM") as ps2tp:
                    for mc, (src, mcnt) in enumerate(ztmp):
                        t = s2.tile([128, 4, NG, 4, 32], mybir.dt.bfloat16,
                                    tag=f"ztr{mc}", name=f"ztr{mc}")
                        nc.vector.memset(t[:, :, NG-1, 2:4, :], 0.0)
                        for g in range(NG):
                            kw = min(4, KC - g * 4)
                            ptb = ps2tp.tile([128, 512], mybir.dt.float32,
                                             tag="ptb")
                            for kk in range(kw):
                                nc.tensor.matmul(
                                    out=ptb[:mcnt, kk*128:(kk+1)*128],
                                    lhsT=src[:, :mcnt, g*4 + kk],
                                    rhs=ident[:], start=True, stop=True)
                            nc.vector.tensor_copy(
                                out=t[:mcnt, :, g, 0:kw, :],
                                in_=ptb[:mcnt, 0:kw*128].rearrange(
                                    "m (k b c) -> m b k c", b=4, c=32))
                        ztr.append((t, mcnt))

                # comp 0 (s): srl rows (b=0) w/ Cre, sim rows (b=1) w/ Cim
                # comp 1 (t): trl rows (b=3) w/ Cre, tim rows (b=2) w/ Cim
                comp_seq = [((0, 0), (1, 1)), ((3, 0), (2, 1))]
                with tc.tile_pool(name="ps2po", bufs=4,
                                  space="PSUM") as ps2po:
                    for comp in range(2):
                        for g in range(NG):
                            po0 = ps2po.tile([128, 360], mybir.dt.float32,
                                             tag="po0")
                            po1 = ps2po.tile([128, 360], mybir.dt.float32,
                                             tag="po1")
                            i = 0
                            for (b, reim) in comp_seq[comp]:
                                for mc, (t, mcnt) in enumerate(ztr):
                                    lhsT = t[:mcnt, b, g]
                                    nc.tensor.matmul(
                                        out=po0[:], lhsT=lhsT,
                                        rhs=ct[:mcnt, reim, mc, 0:360],
                                        start=(i == 0), stop=(i == 5))
                                    nc.tensor.matmul(
                                        out=po1[:], lhsT=lhsT,
                                        rhs=ct[:mcnt, reim, mc, 360:720],
                                        start=(i == 0), stop=(i == 5))
                                    i += 1
                            osb = ob.tile([128, N], mybir.dt.bfloat16,
                                          tag="osb")
                            nc.vector.tensor_copy(out=osb[:, 0:360],
                                                  in_=po0[:])
                            nc.vector.tensor_copy(out=osb[:, 360:720],
                                                  in_=po1[:])
                            kw = min(4, KC - g * 4)
                            # psum rows are (kj, c); alternate HWDGE rings
                            eng = nc.sync if (g % 2 == 0) else nc.scalar
                            eng.dma_start(
                                out=outsh[comp].rearrange(
                                    "c k n -> k c n")[g*4:g*4+kw],
                                in_=osb[:kw*C],
                            )
    nc.compile()
    return nc


def _m_perm():
    """Row order of the m axis as seen by stage 2 (segment-major)."""
    perm = []
    for (ms, mw) in MSEG:
        perm += [mb * MC + ms + ml for mb in range(NC) for ml in range(mw)]
    return np.array(perm)


def _host_prep(x_re, x_im, d0, d1):
    xr0, xr1 = x_re[0, :, 0], x_re[0, :, 1]   # (32, L, M)
    xi0, xi1 = x_im[0, :, 0], x_im[0, :, 1]

    def mkx(blocks):
        x = np.concatenate(blocks, axis=0)            # (128, L, M)
        x = np.transpose(x, (2, 1, 0))                # (M, L, 128)
        xp = np.zeros((MP, LP, 128), BF16)
        xp[:M, :L] = x
        return xp
    X1 = mkx([xr0, xi0, -xi1, -xr1])
    X2 = mkx([-xi1, xr1, xr0, -xi0])
    # xsh[core][p, ml, which, lc, c'] = X{which}[core*MC+ml, lc*128+p, c']
    xv = np.stack([X1, X2], axis=1)                   # (MP, 2, LP, 128)
    xv = xv.reshape(NC, MC, 2, LCH, 128, 128)         # (i, ml, w, lc, p, c)
    xv = np.ascontiguousarray(xv.transpose(0, 4, 1, 2, 3, 5))

    def mkd(d):
        dp = np.zeros((MP, LP, KPP), BF16)
        dp[:M, :L, :K] = np.transpose(d, (0, 2, 1))
        return dp
    D0, D1 = mkd(d0), mkd(d1)
    dv = np.stack([D0, D1], axis=1)                   # (MP, 2, LP, KPP)
    dv = dv.reshape(NC, MC, 2, LCH, 128, KPP)
    dv = np.ascontiguousarray(dv.transpose(0, 4, 1, 2, 3, 5))

    m = np.arange(MP, dtype=np.float64)[:, None]
    n = np.arange(N, dtype=np.float64)[None, :]
    th = 2.0 * np.pi * (m * n) / N
    w = np.full((MP, 1), 2.0); w[0] = 1.0; w[360] = 1.0; w[361:] = 0.0
    Cre = (w * np.cos(th)).astype(np.float32)
    Cim = (-w * np.sin(th)).astype(np.float32)
    Cim[0] = 0.0; Cim[360] = 0.0; Cim[361:] = 0.0
    cv = np.stack([Cre, Cim], axis=1)                 # (MP, 2, N)
    cv = cv[_m_perm()]                                # stage-2 m order
    cv = np.concatenate(
        [cv, np.zeros((LCH * 128 - MP, 2, N), np.float32)], axis=0)
    cv = cv.reshape(LCH, 128, 2, N)
    cv = np.ascontiguousarray(cv.transpose(1, 2, 0, 3)).astype(BF16)
    return xv, dv, cv


def kernel(x_re, x_im, d0, d1):
    if "nc" not in _CACHE:
        _CACHE["nc"] = _build()
    nc = _CACHE["nc"]

    xv, dv, cv = _host_prep(np.asarray(x_re), np.asarray(x_im),
                            np.asarray(d0), np.asarray(d1))
    in_maps = [{"xsh": xv[i], "dsh": dv[i], "ctab": cv} for i in range(NC)]
    res = run_bass_kernel_spmd(nc, in_maps, list(range(NC)))

    out = np.empty((B, C, 2, K, N), np.float32)
    for i in range(NC):
        k0 = i * KC
        k1 = min(K, k0 + KC)
        o = res.results[i]["outsh"].astype(np.float32)  # [2, C, KC, N]
        out[0, :, 0, k0:k1] = o[0, :, :k1-k0]
        out[0, :, 1, k0:k1] = o[1, :, :k1-k0]
    return out
